# revision 18
# baseline (speedup 1.0000x reference)
"""Distance-weighted embedding loss on 8 Trainium2 NeuronCores.

reference:
    gathered = embedding[indices]                      # [B, K, D]
    sq = sum((gathered - emb_batch[:,None,:])**2, -1)  # [B, K]
    loss = sum(sq * attr_sim) / B                      # scalar

Sharding: data-parallel over the batch. Each of the 8 cores handles
B/8 = 512 samples; the embedding table is replicated. Each core reduces
its shard to a single partial sum on-device; the host adds the 8
partials and divides by B (the scalar all-reduce).

v7 device algorithm — norm expansion + TensorE contraction:
    ||g - x||^2 = ||g||^2 - 2<g,x> + ||x||^2
The replicated table is shipped bf16 with a per-row norm column
appended (row = [e_0..e_127 | ||e||^2 | 0], 130 bf16 = 260B), so the
row norms arrive with the gather. Per 128-sample group an extended
x-tile  xe = [-2*x | 1 | 0]  (per-partition, [128,130]) turns ONE DVE
2x multiply per gathered chunk
    prod[p, k, :] = row[p, k, :] * xe[p, :]
                  = [-2*g*x | ||g||^2 | 0]
into everything the reduction needs; a skinny accumulating matmul per
(group, k) on the otherwise idle TensorE
    psum[1, 130] += attr_col[128, 1].T @ prod_k[128, 130]
contracts the attr-weighted sum over samples. sum(psum) is then
sum_{p,k} a*(||g||^2 - 2<g,x>), and the tiny ||x||^2 * sum_k a term is
computed on-device from x directly. No scalar-engine work, no
subtract: the only per-element pass is the single DVE multiply, which
sits well under the ~20us gather-DMA stream — the kernel is
memory-bound on the indirect gather.
"""

import ml_dtypes
import numpy as np

import concourse.bass as bass
import concourse.tile as tile
from concourse import bacc, mybir
from concourse.bass_utils import run_bass_kernel_spmd

F32 = mybir.dt.float32
BF16 = mybir.dt.bfloat16
I32 = mybir.dt.int32

NCORES = 8
D = 128
DP = 130          # padded row: D dims + norm + zero pad
P = 128


def build_program(V: int, S_C: int, K: int):
    """Build the per-core Bass program.

    V: table rows; S_C: samples per core (multiple of 128);
    K: neighbors per sample.
    """
    G = S_C // P
    assert S_C % P == 0

    nc = bacc.Bacc("TRN2", target_bir_lowering=False, debug=False)

    xg_d = nc.dram_tensor("xg", [P, G * D], BF16, kind="ExternalInput")
    attr_d = nc.dram_tensor("attr", [P, G * K], BF16, kind="ExternalInput")
    offs_d = nc.dram_tensor("offsets", [P, G * K], I32, kind="ExternalInput")
    table = nc.dram_tensor("embedding", [V, DP], BF16, kind="ExternalInput")
    loss = nc.dram_tensor("loss", [1, 1], F32, kind="ExternalOutput")

    # chunk schedule: (k0, nct) per group. Small leading chunks start the
    # compute pipeline early; a small final chunk keeps the post-stream
    # chain short.
    first_segs = [(0, 12), (12, 13), (25, 25)]
    wide = [(0, 25), (25, 25)]
    last_segs = [(0, 25), (25, 15), (40, 10)]
    n_mm = G * K

    with tile.TileContext(nc) as tc:
        with (
            tc.tile_pool(name="const", bufs=1) as const,
            tc.tile_pool(name="gather", bufs=8) as gpool,
            tc.tile_pool(name="prod", bufs=6) as ppool,
            tc.tile_pool(name="psum", bufs=1, space="PSUM") as psum,
        ):
            # PE warm-up: ~6us of dummy matmuls during the DMA lead-in lift
            # the HAM clock gate to 2.4 GHz before the real matmuls arrive.
            wones = const.tile([P, 512], BF16)
            nc.vector.memset(wones[:], 1.0)
            wps = psum.tile([1, 512], F32)
            for _ in range(16):
                nc.tensor.matmul(
                    out=wps[:], lhsT=wones[:, :1], rhs=wones[:],
                    start=True, stop=True,
                )

            offs_sb = const.tile([P, G * K], I32)
            nc.sync.dma_start(out=offs_sb[:], in_=offs_d[:])
            xg = const.tile([P, G * D], BF16)
            nc.scalar.dma_start(out=xg[:], in_=xg_d[:])
            attr_sb = const.tile([P, G * K], BF16)
            nc.scalar.dma_start(out=attr_sb[:], in_=attr_d[:])

            # xe[g] = [-2*x_g | 1 | 0], [P, G*DP]
            xe = const.tile([P, G * DP], BF16)
            nc.vector.memset(xe[:], 0.0)
            for g in range(G):
                nc.vector.tensor_scalar_mul(
                    out=xe[:, g * DP:g * DP + D],
                    in0=xg[:, g * D:(g + 1) * D],
                    scalar1=-2.0,
                )
            ones_col = const.tile([P, G], BF16)
            nc.vector.memset(ones_col[:], 1.0)
            nc.vector.tensor_copy(
                out=xe[:].rearrange("p (g d) -> p g d", g=G)[:, :, D:D + 1],
                in_=ones_col[:].unsqueeze(2),
            )

            # term3: sum_p ||x_p||^2 * sum_k attr[p,k]
            x2 = const.tile([P, G * D], BF16)
            nc.vector.tensor_tensor(
                out=x2[:], in0=xg[:], in1=xg[:], op=mybir.AluOpType.mult,
            )
            nx = const.tile([P, G], F32)
            nc.vector.tensor_reduce(
                out=nx[:], in_=x2[:].rearrange("p (g d) -> p g d", g=G),
                axis=mybir.AxisListType.X, op=mybir.AluOpType.add,
            )
            ap_ = const.tile([P, G], F32)
            nc.vector.tensor_reduce(
                out=ap_[:], in_=attr_sb[:].rearrange("p (g k) -> p g k", g=G),
                axis=mybir.AxisListType.X, op=mybir.AluOpType.add,
            )
            t3 = const.tile([P, G], F32)
            nc.vector.tensor_tensor(
                out=t3[:], in0=nx[:], in1=ap_[:], op=mybir.AluOpType.mult,
            )
            t3s = const.tile([P, 1], F32)
            nc.vector.tensor_reduce(
                out=t3s[:], in_=t3[:],
                axis=mybir.AxisListType.X, op=mybir.AluOpType.add,
            )

            acc = psum.tile([1, DP], F32)
            mm_i = 0
            for g in range(G):
                if g == 0:
                    segs = first_segs
                elif g == G - 1:
                    segs = last_segs
                else:
                    segs = wide
                xe_g = xe[:, g * DP:(g + 1) * DP]
                for k0, nct in segs:
                    ioff = bass.IndirectOffsetOnAxis(
                        ap=offs_sb[:, g * K + k0: g * K + k0 + nct],
                        axis=0,
                    )
                    m = gpool.tile([P, 25 * DP], BF16, tag="m")
                    mm = m[:, :nct * DP]
                    nc.gpsimd.indirect_dma_start(
                        out=mm, out_offset=None,
                        in_=table[:], in_offset=ioff,
                    )
                    pt = ppool.tile([P, 25 * DP], BF16, tag="pt")
                    prod = pt[:, :nct * DP]
                    nc.vector.tensor_tensor(
                        out=prod.rearrange("p (n d) -> p n d", n=nct),
                        in0=mm.rearrange("p (n d) -> p n d", n=nct),
                        in1=xe_g.unsqueeze(1).to_broadcast([P, nct, DP]),
                        op=mybir.AluOpType.mult,
                    )
                    for k in range(nct):
                        col = g * K + k0 + k
                        nc.tensor.matmul(
                            out=acc[:],
                            lhsT=attr_sb[:, col:col + 1],
                            rhs=prod[:, k * DP:(k + 1) * DP],
                            start=(mm_i == 0), stop=(mm_i == n_mm - 1),
                        )
                        mm_i += 1
            assert mm_i == n_mm

            # loss = sum(acc) + sum_p t3s[p]
            ones_f = const.tile([P, 1], F32)
            nc.vector.memset(ones_f[:], 1.0)
            t3p = psum.tile([1, 1], F32)
            nc.tensor.matmul(
                out=t3p[:], lhsT=t3s[:], rhs=ones_f[:], start=True, stop=True,
            )
            r1 = const.tile([1, 1], F32)
            nc.vector.tensor_reduce(
                out=r1[:], in_=acc[:],
                axis=mybir.AxisListType.X, op=mybir.AluOpType.add,
            )
            tot = const.tile([1, 1], F32)
            nc.vector.tensor_tensor(
                out=tot[:], in0=r1[:], in1=t3p[:], op=mybir.AluOpType.add,
            )
            nc.sync.dma_start(out=loss[:], in_=tot[:])

    nc.compile()
    return nc


def shard_inputs(emb_batch, embedding, attr_sim, indices, ncores: int = NCORES):
    """Build the per-core input maps (layout/dtype prep only; the
    replicated table is augmented with a per-row norm column)."""
    B, K = attr_sim.shape
    s_c = B // ncores
    g = s_c // P
    xg_all = np.asarray(emb_batch, dtype=np.float32).astype(ml_dtypes.bfloat16)
    attr_bf = np.asarray(attr_sim, dtype=np.float32).astype(ml_dtypes.bfloat16)
    emb32 = np.asarray(embedding, dtype=np.float32)
    V = emb32.shape[0]
    emb_aug = np.zeros((V, DP), dtype=ml_dtypes.bfloat16)
    emb_aug[:, :D] = emb32.astype(ml_dtypes.bfloat16)
    emb_aug[:, D] = np.einsum("vd,vd->v", emb32, emb32).astype(
        ml_dtypes.bfloat16)
    idx = np.asarray(indices).astype(np.int32)

    in_maps = []
    for c in range(ncores):
        sl = slice(c * s_c, (c + 1) * s_c)
        # [s_c, X] -> [P, G*X]: t[p, g*X + x] = src[g*128 + p, x]
        xg = np.ascontiguousarray(
            xg_all[sl].reshape(g, P, D).transpose(1, 0, 2).reshape(P, g * D))
        at = np.ascontiguousarray(
            attr_bf[sl].reshape(g, P, K).transpose(1, 0, 2).reshape(P, g * K))
        offs = np.ascontiguousarray(
            idx[sl].reshape(g, P, K).transpose(1, 0, 2).reshape(P, g * K))
        in_maps.append({
            "xg": xg,
            "attr": at,
            "offsets": offs,
            "embedding": emb_aug,
        })
    return in_maps


_cached = {}


def kernel(emb_batch, embedding, attr_sim, indices, beta):
    emb_batch = np.asarray(emb_batch)
    embedding = np.asarray(embedding)
    attr_sim = np.asarray(attr_sim)
    indices = np.asarray(indices)
    B, K = attr_sim.shape
    V = embedding.shape[0]
    key = (V, B // NCORES, K)
    if key not in _cached:
        _cached[key] = build_program(V, B // NCORES, K)
    nc = _cached[key]
    in_maps = shard_inputs(emb_batch, embedding, attr_sim, indices)
    res = run_bass_kernel_spmd(nc, in_maps, list(range(NCORES)))
    partials = [res.results[c]["loss"][0, 0] for c in range(NCORES)]
    return np.float32(np.sum(np.asarray(partials, dtype=np.float64)) / B)
